# revision 18
# baseline (speedup 1.0000x reference)
"""Trainium2 Bass kernel for nn_Block_46059229282655 (dense transformer block).

Sharding: sequence-parallel over 8 NeuronCores (512 tokens each), weights
replicated.  K/V for both attentions are AllGathered (bf16, packed).  All
activations are kept feature-major ([C_chunk=128 partitions, tokens free]) so
matmuls never need transposes; V is gathered token-major with a baked-in ones
column per head so the softmax denominator falls out of the PV matmul.
"""

import os
from contextlib import ExitStack

import numpy as np
import ml_dtypes

import concourse.bass as bass
import concourse.mybir as mybir
import concourse.tile as tile
from concourse import bacc
from concourse.bass_utils import run_bass_kernel_spmd

BFNP = ml_dtypes.bfloat16
F32 = mybir.dt.float32
BF16 = mybir.dt.bfloat16
AF = mybir.ActivationFunctionType
ALU = mybir.AluOpType

R = 8            # cores
P = 128          # partitions
T = 512          # tokens per core
N = R * T        # 4096 tokens
C = 768
CC = C // P      # 6 channel chunks
NHS, HDS = 12, 64
NHC, HDC = 8, 96
FF = 3072
FC = FF // P     # 24
NKC = N // P     # 32 key chunks
WVS = NHS * (HDS + 1)   # 780
WVC = NHC * (HDC + 1)   # 776
KT_E = C * T            # 393216 elements of a K^T block
VS_E = T * WVS          # 399360
VC_E = T * WVC          # 397312
KVS = KT_E + VS_E
KVC = KT_E + VC_E
EPS = 1e-5

KDBG = bool(os.environ.get("KDBG"))

_BUILT = None


def _build():
    nc = bacc.Bacc(None, target_bir_lowering=False, debug=False)
    dt = mybir.dt

    # ---------------- I/O ----------------
    xT_d = nc.dram_tensor("xT", [C, T], F32, kind="ExternalInput")
    yT_d = nc.dram_tensor("yT", [C, T], BF16, kind="ExternalInput")
    zT_d = nc.dram_tensor("zT", [C, T], BF16, kind="ExternalInput")

    w_q_d = nc.dram_tensor("w_q", [C, NHS * P], BF16, kind="ExternalInput")
    w_k_d = nc.dram_tensor("w_k", [C, C], BF16, kind="ExternalInput")
    w_ve_d = nc.dram_tensor("w_ve", [C, WVS], BF16, kind="ExternalInput")
    vb_e_d = nc.dram_tensor("vb_e", [1, WVS], BF16, kind="ExternalInput")
    w_proj_d = nc.dram_tensor("w_proj", [C, C], BF16, kind="ExternalInput")
    w_caq_d = nc.dram_tensor("w_caq", [C, C], BF16, kind="ExternalInput")
    w_cak_d = nc.dram_tensor("w_cak", [C, C], BF16, kind="ExternalInput")
    w_cave_d = nc.dram_tensor("w_cave", [C, WVC], BF16, kind="ExternalInput")
    vbc_e_d = nc.dram_tensor("vbc_e", [1, WVC], BF16, kind="ExternalInput")
    w_cao_d = nc.dram_tensor("w_cao", [C, C], BF16, kind="ExternalInput")
    w_m2a_d = nc.dram_tensor("w_m2a", [C, FF], BF16, kind="ExternalInput")
    w_m2b_d = nc.dram_tensor("w_m2b", [FF, C], BF16, kind="ExternalInput")
    w_m1a_d = nc.dram_tensor("w_m1a", [C, FF], BF16, kind="ExternalInput")
    w_m1b_d = nc.dram_tensor("w_m1b", [FF, C], BF16, kind="ExternalInput")
    w_pw1_d = nc.dram_tensor("w_pw1", [C, C], BF16, kind="ExternalInput")
    w_pw2_d = nc.dram_tensor("w_pw2", [C, C], BF16, kind="ExternalInput")

    # [parts, k] fp32 vectors (host pre-reshaped (k,parts)->T)
    vec_specs = {
        "ln1_g": (P, CC), "ln1_b": (P, CC), "ln2_g": (P, CC), "ln2_b": (P, CC),
        "qb": (P, NHS), "kb": (P, CC), "projb": (P, CC), "caob": (P, CC),
        "m2b1": (P, FC), "m2b2": (P, CC), "m1b1": (P, FC), "m1b2": (P, CC),
        "pw1b": (P, CC), "pw2b": (P, CC),
    }
    vec_d = {k: nc.dram_tensor(k, list(s), F32, kind="ExternalInput")
             for k, s in vec_specs.items()}

    o_p1 = nc.dram_tensor("o_p1", [C, T], F32, kind="ExternalOutput")
    o_pw1 = nc.dram_tensor("o_pw1", [C, T], F32, kind="ExternalOutput")
    o_pw2 = nc.dram_tensor("o_pw2", [C, T], F32, kind="ExternalOutput")

    kvK_in = nc.dram_tensor("kvK_in", [KT_E], BF16)
    kvK_out = nc.dram_tensor("kvK_out", [R, KT_E], BF16, addr_space="Shared")
    kvV_in = nc.dram_tensor("kvV_in", [VS_E], BF16)
    kvV_out = nc.dram_tensor("kvV_out", [R, VS_E], BF16, addr_space="Shared")
    kvC_in = nc.dram_tensor("kvC_in", [KVC], BF16)
    kvC_out = nc.dram_tensor("kvC_out", [R, KVC], BF16, addr_space="Shared")

    dbg = {}
    if KDBG:
        for nm in ("d_x1", "d_x1f", "d_x2", "d_p2", "d_at", "d_h1"):
            dbg[nm] = nc.dram_tensor(nm, [C, T], F32, kind="ExternalOutput")

    with tile.TileContext(nc) as tc, ExitStack() as top:
        # ------------- global pools -------------
        cpool = top.enter_context(tc.tile_pool(name="consts", bufs=1))
        statp = top.enter_context(tc.tile_pool(name="statp", bufs=1))
        lnp = top.enter_context(tc.tile_pool(name="lnp", bufs=1))
        w66p = top.enter_context(tc.tile_pool(name="w66p", bufs=1))
        ps_st = top.enter_context(tc.tile_pool(name="ps_st", bufs=1, space="PSUM"))
        ps_ot = top.enter_context(tc.tile_pool(name="ps_ot", bufs=1, space="PSUM"))
        ps_mm = top.enter_context(tc.tile_pool(name="ps_mm", bufs=1, space="PSUM"))

        # ------------- constants -------------
        vcc_names = [k for k, s in vec_specs.items() if s == (P, CC)]
        vfc_names = [k for k, s in vec_specs.items() if s == (P, FC)]
        vcc_t = cpool.tile([P, len(vcc_names) * CC], F32, tag="vcc", name="vcc_t")
        vfc_t = cpool.tile([P, len(vfc_names) * FC], F32, tag="vfc", name="vfc_t")
        qb_t = cpool.tile([P, NHS], F32, tag="vqb", name="qb_t")
        nc.sync.dma_start(qb_t[:], vec_d["qb"][:])
        vec = {"qb": qb_t}
        for i, k in enumerate(vcc_names):
            nc.sync.dma_start(vcc_t[:, i * CC:(i + 1) * CC], vec_d[k][:])
            vec[k] = vcc_t[:, i * CC:(i + 1) * CC]
        for i, k in enumerate(vfc_names):
            nc.sync.dma_start(vfc_t[:, i * FC:(i + 1) * FC], vec_d[k][:])
            vec[k] = vfc_t[:, i * FC:(i + 1) * FC]
        vb_sb = cpool.tile([1, WVS], BF16, tag="vb", name="vb_sb")
        nc.sync.dma_start(vb_sb[:], vb_e_d[:])
        vbc_sb = cpool.tile([1, WVC], BF16, tag="vbc", name="vbc_sb")
        nc.sync.dma_start(vbc_sb[:], vbc_e_d[:])
        ones_col = cpool.tile([P, 1], BF16, tag="oc", name="ones_col")
        nc.vector.memset(ones_col[:], 1.0)
        ones_row = cpool.tile([1, P], BF16, tag="or", name="ones_row")
        nc.vector.memset(ones_row[:], 1.0)
        eps_t = cpool.tile([1, 1], F32, tag="eps", name="eps_t")
        nc.vector.memset(eps_t[:], float(EPS))

        # ------------- helpers -------------
        def ln_stats(src, nm):
            """src: [P, CC, T] fp32 SBUF. Returns psum broadcasts (rstd_b, mrstd_b)."""
            if src.dtype == BF16:
                xb = src
            else:
                xb = lnp.tile([P, CC, T], BF16, tag="lnxb", bufs=2, name=f"xb_{nm}")
            sq = lnp.tile([P, CC, T], BF16, tag="lnsq", bufs=2, name=f"sq_{nm}")
            for c in range(CC):
                if xb is not src:
                    nc.vector.tensor_copy(xb[:, c], src[:, c])
                nc.vector.tensor_mul(sq[:, c], xb[:, c], xb[:, c])
            s1 = ps_mm.tile([1, T], F32, tag="mm", bufs=2, name=f"s1_{nm}")
            for c in range(CC):
                nc.tensor.matmul(s1[:], ones_col[:], xb[:, c],
                                 start=(c == 0), stop=(c == CC - 1))
            s2 = ps_mm.tile([1, T], F32, tag="mm", bufs=2, name=f"s2_{nm}")
            for c in range(CC):
                nc.tensor.matmul(s2[:], ones_col[:], sq[:, c],
                                 start=(c == 0), stop=(c == CC - 1))
            m = statp.tile([1, T], F32, tag="stat", bufs=6, name=f"m_{nm}")
            nc.vector.tensor_scalar(m[:], s1[:], 1.0 / C, None, ALU.mult)
            ex2 = statp.tile([1, T], F32, tag="stat", bufs=6, name=f"e2_{nm}")
            nc.vector.tensor_scalar(ex2[:], s2[:], 1.0 / C, None, ALU.mult)
            msq = statp.tile([1, T], F32, tag="stat", bufs=6, name=f"ms_{nm}")
            nc.vector.tensor_mul(msq[:], m[:], m[:])
            var = statp.tile([1, T], F32, tag="stat", bufs=6, name=f"va_{nm}")
            nc.vector.tensor_sub(var[:], ex2[:], msq[:])
            sd = statp.tile([1, T], F32, tag="stat", bufs=6, name=f"sd_{nm}")
            nc.scalar.activation(sd[:], var[:], AF.Sqrt, bias=eps_t[:])
            rstd = statp.tile([1, T], F32, tag="stat", bufs=6, name=f"rs_{nm}")
            nc.vector.reciprocal_approx_fast(rstd[:], sd[:])
            mr = statp.tile([1, T], F32, tag="stat", bufs=6, name=f"mr_{nm}")
            nc.vector.tensor_mul(mr[:], m[:], rstd[:])
            rstd_b = statp.tile([1, T], BF16, tag="statb", bufs=2, name=f"rb_{nm}")
            nc.vector.tensor_copy(rstd_b[:], rstd[:])
            mr_b = statp.tile([1, T], BF16, tag="statb", bufs=2, name=f"mb_{nm}")
            nc.vector.tensor_copy(mr_b[:], mr[:])
            bc_r = ps_st.tile([P, 2 * T], F32, tag="st", bufs=2, name=f"bcr_{nm}")
            nc.tensor.matmul(bc_r[:, 0:T], ones_row[:], rstd_b[:], start=True, stop=True)
            nc.tensor.matmul(bc_r[:, T:2 * T], ones_row[:], mr_b[:], start=True, stop=True)
            return bc_r

        def ln_apply(src, bc, g, b, dst, nm):
            """dst[:, c] = ((src*rstd) - m*rstd) * g + b, bf16 out."""
            for c in range(CC):
                u = lnp.tile([P, T], F32, tag="lnu", bufs=4, name=f"u_{nm}{c}")
                nc.vector.tensor_mul(u[:], src[:, c], bc[:, 0:T])
                nc.vector.tensor_sub(u[:], u[:], bc[:, T:2 * T])
                nc.vector.tensor_scalar(dst[:, c], u[:], g[:, c:c + 1], b[:, c:c + 1],
                                        ALU.mult, ALU.add)

        def load_w66(dram, nm, pool=None, htag="w66", parts=P, hdim=CC):
            wp = pool or w66p
            ap = dram if isinstance(dram, bass.AP) else dram[:]
            wt = wp.tile([parts, hdim, ap.shape[-1]], BF16, tag=htag, bufs=2,
                         name=f"w_{nm}")
            nc.sync.dma_start(wt[:], ap.rearrange("(a p) n -> p a n", p=parts))
            return wt

        def linear_fm(dst, src, w_sb, bias, func, nm, cin=CC, dout=CC):
            """dst [P, dout, T] <- act(W^T @ src + bias); w_sb [P, cin, dout*128]."""
            for d in range(dout):
                ps = ps_mm.tile([P, T], F32, tag="mm", bufs=2, name=f"p_{nm}{d}")
                for c in range(cin):
                    nc.tensor.matmul(ps[:], w_sb[:, c, d * P:(d + 1) * P], src[:, c],
                                     start=(c == 0), stop=(c == cin - 1))
                if func is None:
                    nc.scalar.copy(dst[:, d], ps[:])
                else:
                    nc.scalar.activation(dst[:, d], ps[:], func,
                                         bias=bias[:, d:d + 1])

        def linear_resid(dst, src, w_sb, bias, resid, nm, cin=CC):
            for d in range(CC):
                ps = ps_mm.tile([P, T], F32, tag="mm", bufs=2, name=f"pr_{nm}{d}")
                for c in range(cin):
                    nc.tensor.matmul(ps[:], w_sb[:, c, d * P:(d + 1) * P], src[:, c],
                                     start=(c == 0), stop=(c == cin - 1))
                u = lnp.tile([P, T], F32, tag="lnu", bufs=4, name=f"t_{nm}{d}")
                nc.scalar.activation(u[:], ps[:], AF.Identity, bias=bias[:, d:d + 1])
                nc.vector.tensor_add(dst[:, d], u[:], resid[:, d])

        def v_tokmajor(dst, src, wv_sb, vbias, width, nm):
            """dst [P, 4, width] token-major V (+ones cols)."""
            half = width // 2
            for tt in range(4):
                for hh in range(2):
                    ps = ps_mm.tile([P, half], F32, tag="mm", bufs=2,
                                    name=f"v_{nm}{tt}{hh}")
                    for c in range(CC):
                        nc.tensor.matmul(ps[:], src[:, c, tt * P:(tt + 1) * P],
                                         wv_sb[:, c, hh * half:(hh + 1) * half],
                                         start=(c == 0), stop=False)
                    nc.tensor.matmul(ps[:], ones_row[:],
                                     vbias[:, hh * half:(hh + 1) * half],
                                     start=False, stop=True)
                    nc.scalar.copy(dst[:, tt, hh * half:(hh + 1) * half], ps[:])

        def mlp_first(dst, src, w_dram, bias, pool, nm):
            # dst [P, FC, T] = gelu(src @ W1 + b1); W1 [C, FF] streamed in halves
            FH = FC // 2
            for half in range(2):
                wt = pool.tile([P, CC, FH * P], BF16, tag="wma", bufs=2,
                               name=f"wma_{nm}{half}")
                nc.sync.dma_start(
                    wt[:], w_dram[:, half * FH * P:(half + 1) * FH * P]
                    .rearrange("(a p) n -> p a n", p=P))
                for f in range(FH):
                    fo = half * FH + f
                    ps = ps_mm.tile([P, T], F32, tag="mm", bufs=2,
                                    name=f"pm_{nm}{fo}")
                    for c in range(CC):
                        nc.tensor.matmul(ps[:], wt[:, c, f * P:(f + 1) * P],
                                         src[:, c], start=(c == 0),
                                         stop=(c == CC - 1))
                    nc.scalar.activation(dst[:, fo], ps[:], AF.Gelu,
                                         bias=bias[:, fo:fo + 1])

        def mlp_second(dst, src, w_dram, bias, resid, pool, nm):
            # dst [P, CC, T] = src @ W2 + b2 + resid; W2 [FF, C] streamed in halves
            FH = FC // 2
            wts = []
            for half in range(2):
                wt = pool.tile([P, FH, C], BF16, tag="wmb", bufs=2,
                               name=f"wmb_{nm}{half}")
                nc.sync.dma_start(
                    wt[:], w_dram[half * FH * P:(half + 1) * FH * P, :]
                    .rearrange("(a p) n -> p a n", p=P))
                wts.append(wt)
            for d in range(CC):
                ps = ps_mm.tile([P, T], F32, tag="mm", bufs=2, name=f"pr_{nm}{d}")
                for c in range(FC):
                    wt = wts[c // FH]
                    nc.tensor.matmul(ps[:], wt[:, c % FH, d * P:(d + 1) * P],
                                     src[:, c], start=(c == 0), stop=(c == FC - 1))
                u = lnp.tile([P, T], F32, tag="lnu", bufs=4, name=f"t_{nm}{d}")
                nc.scalar.activation(u[:], ps[:], AF.Identity, bias=bias[:, d:d + 1])
                nc.vector.tensor_add(dst[:, d], u[:], resid[:, d])

        def tap(nm, src):
            if KDBG and nm in dbg:
                for c in range(CC):
                    nc.gpsimd.dma_start(
                        dbg[nm][:].rearrange("(a p) n -> p a n", p=P)[:, c], src[:, c])

        # ===================== phase A =====================
        es_x = ExitStack()
        pgx = es_x.enter_context(tc.tile_pool(name="pgx", bufs=1, side="left"))
        es_kv = ExitStack()
        pgkv = es_kv.enter_context(tc.tile_pool(name="pgkv", bufs=1, side="left"))
        es_a = ExitStack()
        pga = es_a.enter_context(tc.tile_pool(name="pga", bufs=1, side="left"))

        xT = pgx.tile([P, CC, T], F32, tag="xT", name="xT_sb")
        for c in range(CC):
            nc.sync.dma_start(
                xT[:, c], xT_d[:].rearrange("(a p) n -> p a n", p=P)[:, c])

        h1 = pga.tile([P, CC, T], BF16, tag="h1", name="h1_sb")
        bc = ln_stats(xT, "lx")
        ln_apply(xT, bc, vec["ln1_g"], vec["ln1_b"], h1, "lx")

        KTl = pga.tile([P, CC, T], BF16, tag="KTl", name="KTl_sb")
        wk = load_w66(w_k_d, "wk")
        linear_fm(KTl, h1, wk, vec["kb"], AF.Identity, "kt")
        nc.sync.dma_start(
            kvK_in[:].rearrange("(a p n) -> p a n", p=P, n=T), KTl[:])
        nc.gpsimd.collective_compute(
            "AllGather", ALU.bypass, replica_groups=[list(range(R))],
            ins=[kvK_in[:]], outs=[kvK_out[:]])

        wv = pga.tile([P, CC, WVS], BF16, tag="wv", name="wv_sb")
        nc.sync.dma_start(wv[:], w_ve_d[:].rearrange("(a p) n -> p a n", p=P))
        Vl = pga.tile([P, 4, WVS], BF16, tag="Vl", name="Vl_sb")
        v_tokmajor(Vl, h1, wv, vb_sb, WVS, "vs")
        nc.sync.dma_start(
            kvV_in[:].rearrange("(a p n) -> p a n", p=P, n=WVS), Vl[:])
        nc.gpsimd.collective_compute(
            "AllGather", ALU.bypass, replica_groups=[list(range(R))],
            ins=[kvV_in[:]], outs=[kvV_out[:]])

        # Q (duplicated across both partition halves for 2-packed scores)
        QT = pgx.tile([P, NHS, T], BF16, tag="QT", name="QT_sb")
        wq0 = load_w66(w_q_d[:, 0:CC * P], "wq0")
        wq1 = load_w66(w_q_d[:, CC * P:2 * CC * P], "wq1")
        for h in range(NHS):
            wt = wq0 if h < CC else wq1
            ps = ps_mm.tile([P, T], F32, tag="mm", bufs=2, name=f"q{h}")
            for c in range(CC):
                nc.tensor.matmul(ps[:], wt[:, c, (h % CC) * P:(h % CC + 1) * P],
                                 h1[:, c], start=(c == 0), stop=(c == CC - 1))
            nc.scalar.activation(QT[:, h], ps[:], AF.Identity,
                                 bias=vec["qb"][:, h:h + 1])

        # cross-attention K/V from y, z (overlaps the AllGather above)
        yT = pga.tile([P, CC, T], BF16, tag="yT", name="yT_sb")
        for c in range(CC):
            nc.sync.dma_start(
                yT[:, c], yT_d[:].rearrange("(a p) n -> p a n", p=P)[:, c])
        hy = pga.tile([P, CC, T], BF16, tag="hyz", bufs=2, name="hy_sb")
        bcy = ln_stats(yT, "ly")
        ln_apply(yT, bcy, vec["ln1_g"], vec["ln1_b"], hy, "ly")
        KcT = pgkv.tile([HDC, NHC, T], BF16, tag="KcT", name="KcT_sb")
        wcak = load_w66(w_cak_d, "wcak")
        for h in range(NHC):
            ps = ps_mm.tile([HDC, T], F32, tag="mm", bufs=2, name=f"kc{h}")
            for c in range(CC):
                nc.tensor.matmul(ps[:], wcak[:, c, HDC * h:HDC * (h + 1)], hy[:, c],
                                 start=(c == 0), stop=(c == CC - 1))
            nc.scalar.copy(KcT[:, h], ps[:])

        zT = pga.tile([P, CC, T], BF16, tag="zT", name="zT_sb")
        for c in range(CC):
            nc.sync.dma_start(
                zT[:, c], zT_d[:].rearrange("(a p) n -> p a n", p=P)[:, c])
        hz = pga.tile([P, CC, T], BF16, tag="hyz", bufs=2, name="hz_sb")
        bcz = ln_stats(zT, "lz")
        ln_apply(zT, bcz, vec["ln1_g"], vec["ln1_b"], hz, "lz")
        wvc = pga.tile([P, CC, WVC], BF16, tag="wvc", name="wvc_sb")
        nc.sync.dma_start(wvc[:], w_cave_d[:].rearrange("(a p) n -> p a n", p=P))
        Vcl = pgkv.tile([P, 4, WVC], BF16, tag="Vcl", name="Vcl_sb")
        v_tokmajor(Vcl, hz, wvc, vbc_sb, WVC, "vc")
        tap("d_h1", h1)
        es_a.close()

        # ===================== phase B: self-attention =====================
        es_x1 = ExitStack()
        pgx1 = es_x1.enter_context(tc.tile_pool(name="pgx1", bufs=1, side="right"))
        es_b = ExitStack()
        pgb = es_b.enter_context(tc.tile_pool(name="pgb", bufs=1, side="right"))
        ktp = pgb
        exp_p = pgb
        atp = pgb

        Vfull = pgb.tile([P, NKC, WVS], BF16, tag="Vfull", name="Vfull_sb")
        for r in range(R):
            nc.gpsimd.dma_start(
                Vfull[:, 4 * r:4 * (r + 1), :],
                kvV_out[r, :].rearrange("(a p n) -> p a n", p=P, n=WVS))

        AT = atp.tile([P, CC, T], BF16, tag="at", name="AT_self")
        sc_s = float(HDS) ** -0.5
        NPR = NKC // 2  # 16 chunk-pairs
        for h in range(NHS):
            # packed K^T: partitions 0-63 = even chunk, 64-127 = odd chunk
            kt = ktp.tile([P, NPR, P], BF16, tag="kt", bufs=2, name=f"ktS{h}")
            for r in range(R):
                src = kvK_out[r, HDS * h * T:(HDS * h + HDS) * T].rearrange(
                    "(p a b n) -> p a b n", p=HDS, a=2, b=2, n=P)
                nc.sync.dma_start(kt[0:HDS, 2 * r:2 * r + 2, :], src[:, :, 0, :])
                nc.sync.dma_start(kt[HDS:P, 2 * r:2 * r + 2, :], src[:, :, 1, :])
            ot = ps_ot.tile([HDS + 1, T], F32, tag="ot", bufs=2, name=f"otS{h}")
            for b2 in range(NPR):
                st = ps_st.tile([P, 2 * T], F32, tag="st", bufs=2, name=f"stS{h}_{b2}")
                nc.tensor.matmul(st[:, 0:T], kt[0:HDS, b2, :], QT[0:HDS, h, :],
                                 start=True, stop=True)
                nc.tensor.matmul(st[:, T:2 * T], kt[HDS:P, b2, :], QT[HDS:P, h, :],
                                 start=True, stop=True)
                ex = exp_p.tile([P, 2 * T], BF16, tag="ex", bufs=3, name=f"exS{h}_{b2}")
                nc.scalar.activation(ex[:], st[:], AF.Exp, scale=sc_s)
                for u in range(2):
                    j = 2 * b2 + u
                    nc.tensor.matmul(ot[:], Vfull[:, j, 65 * h:65 * h + 65],
                                     ex[:, u * T:(u + 1) * T],
                                     start=(j == 0), stop=(j == NKC - 1),
                                     skip_group_check=True)
            denr = statp.tile([1, T], F32, tag="stat", bufs=6, name=f"denrS{h}")
            nc.vector.tensor_copy(denr[:], ot[HDS:HDS + 1, :])
            den = statp.tile([1, T], F32, tag="stat", bufs=6, name=f"denS{h}")
            nc.vector.reciprocal_approx_fast(den[:], denr[:])
            bcd = lnp.tile([HDS, T], F32, tag="bcd", bufs=2, name=f"bcdS{h}")
            nc.gpsimd.partition_broadcast(bcd[:], den[:])
            osb = lnp.tile([HDS, T], F32, tag="osb", bufs=2, name=f"osbS{h}")
            nc.vector.tensor_copy(osb[:], ot[0:HDS, :])
            nc.vector.tensor_mul(AT[HDS * (h % 2):HDS * (h % 2) + HDS, h // 2, :],
                                 osb[:], bcd[:])
            if h == 1:
                # export cross K/V + launch its AllGather while self-attn runs
                nc.sync.dma_start(
                    kvC_in[0:KT_E].rearrange("(p a n) -> p a n", p=HDC, n=T),
                    KcT[:])
                nc.sync.dma_start(
                    kvC_in[KT_E:KVC].rearrange("(a p n) -> p a n", p=P, n=WVC),
                    Vcl[:])
                nc.gpsimd.collective_compute(
                    "AllGather", ALU.bypass, replica_groups=[list(range(R))],
                    ins=[kvC_in[:]], outs=[kvC_out[:]])
                es_kv.close()

        # proj + residual -> x1
        x1 = pgx1.tile([P, CC, T], F32, tag="x1", name="x1_sb")
        wpj = load_w66(w_proj_d, "wpj")
        linear_resid(x1, AT, wpj, vec["projb"], xT, "pj")
        tap("d_x1", x1)
        es_x.close()
        es_b.close()

        # ===================== phase C: MLP2 =====================
        es_x1f = ExitStack()
        pgx1f = es_x1f.enter_context(tc.tile_pool(name="pgx1f", bufs=1, side="left"))
        es_c = ExitStack()
        pgc = es_c.enter_context(tc.tile_pool(name="pgc", bufs=1, side="left"))

        h2 = pgc.tile([P, CC, T], BF16, tag="h2", name="h2_sb")
        bc1 = ln_stats(x1, "l1")
        ln_apply(x1, bc1, vec["ln2_g"], vec["ln2_b"], h2, "l1")

        HT = pgc.tile([P, FC, T], BF16, tag="ht", name="HT2_sb")
        mlp_first(HT, h2, w_m2a_d, vec["m2b1"], pgc, "m2a")
        x1f = pgx1f.tile([P, CC, T], F32, tag="x1f", name="x1f_sb")
        mlp_second(x1f, HT, w_m2b_d, vec["m2b2"], x1, pgc, "m2b")

        for c in range(CC):
            nc.sync.dma_start(
                o_p1[:].rearrange("(a p) n -> p a n", p=P)[:, c], x1f[:, c])
        tap("d_x1f", x1f)
        es_x1.close()
        es_c.close()

        # ===================== phase D: cross-attention =====================
        es_x2 = ExitStack()
        pgx2 = es_x2.enter_context(tc.tile_pool(name="pgx2", bufs=1, side="right"))
        es_d = ExitStack()
        pgd = es_d.enter_context(tc.tile_pool(name="pgd", bufs=1, side="right"))
        ktp = pgd
        exp_p = pgd
        atp = pgd

        hq = pgd.tile([P, CC, T], BF16, tag="hq", name="hq_sb")
        bcq = ln_stats(x1f, "lq")
        ln_apply(x1f, bcq, vec["ln1_g"], vec["ln1_b"], hq, "lq")

        QcT = pgd.tile([HDC, NHC, T], BF16, tag="QcT", name="QcT_sb")
        wcaq = load_w66(w_caq_d, "wcaq")
        for h in range(NHC):
            ps = ps_mm.tile([HDC, T], F32, tag="mm", bufs=2, name=f"qc{h}")
            for c in range(CC):
                nc.tensor.matmul(ps[:], wcaq[:, c, HDC * h:HDC * (h + 1)], hq[:, c],
                                 start=(c == 0), stop=(c == CC - 1))
            nc.vector.tensor_copy(QcT[:, h], ps[:])

        Vcfull = pgd.tile([P, NKC, WVC], BF16, tag="Vcfull", name="Vcfull_sb")
        for r in range(R):
            nc.gpsimd.dma_start(
                Vcfull[:, 4 * r:4 * (r + 1), :],
                kvC_out[r, KT_E:KVC].rearrange("(a p n) -> p a n", p=P, n=WVC))

        AcT = atp.tile([HDC, NHC, T], BF16, tag="atc", name="AT_cross")
        sc_c = float(HDC) ** -0.5
        for h in range(NHC):
            kt = ktp.tile([HDC, NKC, P], BF16, tag="kt", bufs=2, name=f"ktC{h}")
            for r in range(R):
                nc.sync.dma_start(
                    kt[:, 4 * r:4 * (r + 1), :],
                    kvC_out[r, 0:KT_E]
                    .rearrange("(p a n) -> p a n", p=HDC, n=T)[:, h, :]
                    .rearrange("p (j n) -> p j n", n=P))
            qrhs = QcT[:, h, :]
            ot = ps_ot.tile([HDC + 1, T], F32, tag="ot", bufs=2, name=f"otC{h}")
            for b2 in range(NKC // 2):
                st = ps_st.tile([P, 2 * T], F32, tag="st", bufs=2, name=f"stC{h}_{b2}")
                for u in range(2):
                    j = 2 * b2 + u
                    nc.tensor.matmul(st[:, u * T:(u + 1) * T], kt[:, j, :], qrhs,
                                     start=True, stop=True)
                ex = exp_p.tile([P, 2 * T], BF16, tag="ex", bufs=3, name=f"exC{h}_{b2}")
                nc.scalar.activation(ex[:], st[:], AF.Exp, scale=sc_c)
                for u in range(2):
                    j = 2 * b2 + u
                    nc.tensor.matmul(ot[:], Vcfull[:, j, 97 * h:97 * h + 97],
                                     ex[:, u * T:(u + 1) * T],
                                     start=(j == 0), stop=(j == NKC - 1),
                                     skip_group_check=True)
            denr = statp.tile([1, T], F32, tag="stat", bufs=6, name=f"denrC{h}")
            nc.vector.tensor_copy(denr[:], ot[HDC:HDC + 1, :])
            den = statp.tile([1, T], F32, tag="stat", bufs=6, name=f"denC{h}")
            nc.vector.reciprocal_approx_fast(den[:], denr[:])
            bcd = lnp.tile([HDC, T], F32, tag="bcd", bufs=2, name=f"bcdC{h}")
            nc.gpsimd.partition_broadcast(bcd[:], den[:])
            osb = lnp.tile([HDC, T], F32, tag="osb", bufs=2, name=f"osbC{h}")
            nc.vector.tensor_copy(osb[:], ot[0:HDC, :])
            nc.vector.tensor_mul(AcT[:, h, :], osb[:], bcd[:])

        # ca_o + residual -> x2
        x2 = pgx2.tile([P, CC, T], F32, tag="x2", name="x2_sb")
        wcao = pgd.tile([HDC, NHC, C], BF16, tag="wcao", name="wcao_sb")
        nc.sync.dma_start(wcao[:], w_cao_d[:].rearrange("(a p) n -> p a n", p=HDC))
        for d in range(CC):
            ps = ps_mm.tile([P, T], F32, tag="mm", bufs=2, name=f"cao{d}")
            for h in range(NHC):
                nc.tensor.matmul(ps[:], wcao[:, h, d * P:(d + 1) * P], AcT[:, h, :],
                                 start=(h == 0), stop=(h == NHC - 1))
            u = lnp.tile([P, T], F32, tag="lnu", bufs=4, name=f"tcao{d}")
            nc.scalar.activation(u[:], ps[:], AF.Identity, bias=vec["caob"][:, d:d + 1])
            nc.vector.tensor_add(x2[:, d], u[:], x1f[:, d])
        tap("d_x2", x2)
        es_x1f.close()
        es_d.close()

        # ===================== phase E: MLP + pw heads =====================
        es_e = ExitStack()
        pge = es_e.enter_context(tc.tile_pool(name="pge", bufs=1, side="left"))

        h3 = pge.tile([P, CC, T], BF16, tag="h3", name="h3_sb")
        bc2 = ln_stats(x2, "l2")
        ln_apply(x2, bc2, vec["ln2_g"], vec["ln2_b"], h3, "l2")

        HT1 = pge.tile([P, FC, T], BF16, tag="ht", name="HT1_sb")
        mlp_first(HT1, h3, w_m1a_d, vec["m1b1"], pge, "m1a")
        p2 = pge.tile([P, CC, T], F32, tag="p2", name="p2_sb")
        mlp_second(p2, HT1, w_m1b_d, vec["m1b2"], x2, pge, "m1b")
        tap("d_p2", p2)

        p2b = lnp.tile([P, CC, T], BF16, tag="lnxb", bufs=2, name="p2b_sb")
        for c in range(CC):
            nc.vector.tensor_copy(p2b[:, c], p2[:, c])

        for w_d, bias, out_d, nm in ((w_pw1_d, "pw1b", o_pw1, "pw1"),
                                     (w_pw2_d, "pw2b", o_pw2, "pw2")):
            wt = load_w66(w_d, nm)
            for d in range(CC):
                ps = ps_mm.tile([P, T], F32, tag="mm", bufs=2, name=f"p_{nm}{d}")
                for c in range(CC):
                    nc.tensor.matmul(ps[:], wt[:, c, d * P:(d + 1) * P], p2b[:, c],
                                     start=(c == 0), stop=(c == CC - 1))
                u = lnp.tile([P, T], F32, tag="lnu", bufs=4, name=f"o_{nm}{d}")
                nc.scalar.activation(u[:], ps[:], AF.Gelu, bias=vec[bias][:, d:d + 1])
                nc.sync.dma_start(
                    out_d[:].rearrange("(a p) n -> p a n", p=P)[:, d], u[:])
        es_x2.close()
        es_e.close()

    nc.finalize()
    return nc


def _prep_inputs(inputs):
    f32 = np.float32

    def bf(a):
        return np.ascontiguousarray(a).astype(BFNP)

    def vec128(v, w):
        return np.ascontiguousarray(np.asarray(v, f32).reshape(w, P).T)

    x = np.asarray(inputs["x"], f32).reshape(N, C)
    y = np.asarray(inputs["y"], f32).reshape(N, C)
    z = np.asarray(inputs["z"], f32).reshape(N, C)
    xT = np.ascontiguousarray(x.T)
    yT = np.ascontiguousarray(y.T)
    zT = np.ascontiguousarray(z.T)

    qkv_w = np.asarray(inputs["qkv_w"], f32)
    qkv_b = np.asarray(inputs["qkv_b"], f32)
    w_q = np.zeros((C, NHS * P), f32)
    for h in range(NHS):
        w_q[:, P * h:P * h + HDS] = qkv_w[:, HDS * h:HDS * h + HDS]
        w_q[:, P * h + HDS:P * (h + 1)] = qkv_w[:, HDS * h:HDS * h + HDS]
    w_q = bf(w_q)
    w_k = bf(qkv_w[:, C:2 * C])
    w_v = qkv_w[:, 2 * C:3 * C]
    w_ve = np.zeros((C, WVS), f32)
    vb_e = np.zeros((1, WVS), f32)
    for h in range(NHS):
        w_ve[:, 65 * h:65 * h + 64] = w_v[:, 64 * h:64 * h + 64]
        vb_e[0, 65 * h:65 * h + 64] = qkv_b[2 * C + 64 * h:2 * C + 64 * h + 64]
        vb_e[0, 65 * h + 64] = 1.0

    ca_v = np.asarray(inputs["ca_v_w"], f32)
    w_cave = np.zeros((C, WVC), f32)
    vbc_e = np.zeros((1, WVC), f32)
    for h in range(NHC):
        w_cave[:, 97 * h:97 * h + 96] = ca_v[:, 96 * h:96 * h + 96]
        vbc_e[0, 97 * h + 96] = 1.0

    common = {
        "w_q": w_q, "w_k": w_k, "w_ve": bf(w_ve), "vb_e": bf(vb_e),
        "w_proj": bf(inputs["proj_w"]),
        "w_caq": bf(inputs["ca_q_w"]), "w_cak": bf(inputs["ca_k_w"]),
        "w_cave": bf(w_cave), "vbc_e": bf(vbc_e),
        "w_cao": bf(inputs["ca_o_w"]),
        "w_m2a": bf(inputs["mlp2_w1"]), "w_m2b": bf(inputs["mlp2_w2"]),
        "w_m1a": bf(inputs["mlp_w1"]), "w_m1b": bf(inputs["mlp_w2"]),
        "ln1_g": vec128(inputs["ln1_g"], CC), "ln1_b": vec128(inputs["ln1_b"], CC),
        "ln2_g": vec128(inputs["ln2_g"], CC), "ln2_b": vec128(inputs["ln2_b"], CC),
        "qb": np.ascontiguousarray(np.tile(
            np.asarray(qkv_b[0:C], f32).reshape(NHS, HDS), 2).T),
        "kb": vec128(qkv_b[C:2 * C], CC),
        "projb": vec128(inputs["proj_b"], CC), "caob": vec128(inputs["ca_o_b"], CC),
        "m2b1": vec128(inputs["mlp2_b1"], FC), "m2b2": vec128(inputs["mlp2_b2"], CC),
        "m1b1": vec128(inputs["mlp_b1"], FC), "m1b2": vec128(inputs["mlp_b2"], CC),
    }
    for nm in ("pw1", "pw2"):
        w = np.asarray(inputs[nm + "_w"], f32)
        g = np.asarray(inputs[nm + "_bn_g"], f32)
        b = np.asarray(inputs[nm + "_bn_b"], f32)
        m = np.asarray(inputs[nm + "_bn_m"], f32)
        v = np.asarray(inputs[nm + "_bn_v"], f32)
        scale = g / np.sqrt(v + EPS)
        common["w_" + nm] = bf((w * scale[:, None]).T)
        common[nm + "b"] = vec128(b - m * scale, CC)

    in_maps = []
    for r in range(R):
        m_ = dict(common)
        m_["xT"] = np.ascontiguousarray(xT[:, r * T:(r + 1) * T])
        m_["yT"] = np.ascontiguousarray(yT[:, r * T:(r + 1) * T]).astype(BFNP)
        m_["zT"] = np.ascontiguousarray(zT[:, r * T:(r + 1) * T]).astype(BFNP)
        in_maps.append(m_)
    return in_maps


def _run(inputs, trace=False):
    global _BUILT
    if _BUILT is None:
        _BUILT = _build()
    nc = _BUILT
    in_maps = _prep_inputs(inputs)
    res = run_bass_kernel_spmd(nc, in_maps, core_ids=list(range(R)), trace=trace)

    def gather(name):
        full = np.concatenate([res.results[r][name] for r in range(R)], axis=1)
        return np.ascontiguousarray(full.T).reshape(1, 64, 64, C)

    outs = (gather("o_p1"), gather("o_pw1"), gather("o_pw2"))
    return outs, res


def kernel(**inputs):
    outs, _ = _run(inputs, trace=False)
    return outs


# revision 21
# speedup vs baseline: 1.0145x; 1.0145x over previous
"""Trainium2 Bass kernel for nn_Block_46059229282655 (dense transformer block).

Sharding: sequence-parallel over 8 NeuronCores (512 tokens each), weights
replicated.  K/V for both attentions are AllGathered (bf16, packed).  All
activations are kept feature-major ([C_chunk=128 partitions, tokens free]) so
matmuls never need transposes; V is gathered token-major with a baked-in ones
column per head so the softmax denominator falls out of the PV matmul.
"""

import os
from contextlib import ExitStack

import numpy as np
import ml_dtypes

import concourse.bass as bass
import concourse.mybir as mybir
import concourse.tile as tile
from concourse import bacc
from concourse.bass_utils import run_bass_kernel_spmd

BFNP = ml_dtypes.bfloat16
F32 = mybir.dt.float32
BF16 = mybir.dt.bfloat16
AF = mybir.ActivationFunctionType
ALU = mybir.AluOpType

R = 8            # cores
P = 128          # partitions
T = 512          # tokens per core
N = R * T        # 4096 tokens
C = 768
CC = C // P      # 6 channel chunks
NHS, HDS = 12, 64
NHC, HDC = 8, 96
FF = 3072
FC = FF // P     # 24
NKC = N // P     # 32 key chunks
WVS = NHS * (HDS + 1)   # 780
WVC = NHC * (HDC + 1)   # 776
KT_E = C * T            # 393216 elements of a K^T block
VS_E = T * WVS          # 399360
VC_E = T * WVC          # 397312
KVS = KT_E + VS_E
KVC = KT_E + VC_E
EPS = 1e-5

KDBG = bool(os.environ.get("KDBG"))

_BUILT = None


def _build():
    nc = bacc.Bacc(None, target_bir_lowering=False, debug=False)
    dt = mybir.dt

    # ---------------- I/O ----------------
    xT_d = nc.dram_tensor("xT", [C, T], F32, kind="ExternalInput")
    yT_d = nc.dram_tensor("yT", [C, T], BF16, kind="ExternalInput")
    zT_d = nc.dram_tensor("zT", [C, T], BF16, kind="ExternalInput")

    w_q_d = nc.dram_tensor("w_q", [C, NHS * P], BF16, kind="ExternalInput")
    w_k_d = nc.dram_tensor("w_k", [C, C], BF16, kind="ExternalInput")
    w_ve_d = nc.dram_tensor("w_ve", [C, WVS], BF16, kind="ExternalInput")
    vb_e_d = nc.dram_tensor("vb_e", [1, WVS], BF16, kind="ExternalInput")
    w_proj_d = nc.dram_tensor("w_proj", [C, C], BF16, kind="ExternalInput")
    w_caq_d = nc.dram_tensor("w_caq", [C, C], BF16, kind="ExternalInput")
    w_cak_d = nc.dram_tensor("w_cak", [C, C], BF16, kind="ExternalInput")
    w_cave_d = nc.dram_tensor("w_cave", [C, WVC], BF16, kind="ExternalInput")
    vbc_e_d = nc.dram_tensor("vbc_e", [1, WVC], BF16, kind="ExternalInput")
    w_cao_d = nc.dram_tensor("w_cao", [C, C], BF16, kind="ExternalInput")
    w_m2a_d = nc.dram_tensor("w_m2a", [C, FF], BF16, kind="ExternalInput")
    w_m2b_d = nc.dram_tensor("w_m2b", [FF, C], BF16, kind="ExternalInput")
    w_m1a_d = nc.dram_tensor("w_m1a", [C, FF], BF16, kind="ExternalInput")
    w_m1b_d = nc.dram_tensor("w_m1b", [FF, C], BF16, kind="ExternalInput")
    w_pw1_d = nc.dram_tensor("w_pw1", [C, C], BF16, kind="ExternalInput")
    w_pw2_d = nc.dram_tensor("w_pw2", [C, C], BF16, kind="ExternalInput")

    # [parts, k] fp32 vectors (host pre-reshaped (k,parts)->T)
    vec_specs = {
        "ln1_g": (P, CC), "ln1_b": (P, CC), "ln2_g": (P, CC), "ln2_b": (P, CC),
        "qb": (P, NHS), "kb": (P, CC), "projb": (P, CC), "caob": (P, CC),
        "m2b1": (P, FC), "m2b2": (P, CC), "m1b1": (P, FC), "m1b2": (P, CC),
        "pw1b": (P, CC), "pw2b": (P, CC),
    }
    vec_d = {k: nc.dram_tensor(k, list(s), F32, kind="ExternalInput")
             for k, s in vec_specs.items()}

    o_p1 = nc.dram_tensor("o_p1", [C, T], F32, kind="ExternalOutput")
    o_pw1 = nc.dram_tensor("o_pw1", [C, T], F32, kind="ExternalOutput")
    o_pw2 = nc.dram_tensor("o_pw2", [C, T], F32, kind="ExternalOutput")

    kvK_in = nc.dram_tensor("kvK_in", [KT_E], BF16)
    kvK_out = nc.dram_tensor("kvK_out", [R, KT_E], BF16, addr_space="Shared")
    kvV_in = nc.dram_tensor("kvV_in", [VS_E], BF16)
    kvV_out = nc.dram_tensor("kvV_out", [R, VS_E], BF16, addr_space="Shared")
    kvC_in = nc.dram_tensor("kvC_in", [KVC], BF16)
    kvC_out = nc.dram_tensor("kvC_out", [R, KVC], BF16, addr_space="Shared")

    dbg = {}
    if KDBG:
        for nm in ("d_x1", "d_x1f", "d_x2", "d_p2", "d_at", "d_h1"):
            dbg[nm] = nc.dram_tensor(nm, [C, T], F32, kind="ExternalOutput")

    with tile.TileContext(nc) as tc, ExitStack() as top:
        # ------------- global pools -------------
        cpool = top.enter_context(tc.tile_pool(name="consts", bufs=1))
        statp = top.enter_context(tc.tile_pool(name="statp", bufs=1))
        lnp = top.enter_context(tc.tile_pool(name="lnp", bufs=1))
        w66p = top.enter_context(tc.tile_pool(name="w66p", bufs=1))
        ps_st = top.enter_context(tc.tile_pool(name="ps_st", bufs=1, space="PSUM"))
        ps_ot = top.enter_context(tc.tile_pool(name="ps_ot", bufs=1, space="PSUM"))
        ps_mm = top.enter_context(tc.tile_pool(name="ps_mm", bufs=1, space="PSUM"))

        # ------------- constants -------------
        vcc_names = [k for k, s in vec_specs.items() if s == (P, CC)]
        vfc_names = [k for k, s in vec_specs.items() if s == (P, FC)]
        vcc_t = cpool.tile([P, len(vcc_names) * CC], F32, tag="vcc", name="vcc_t")
        vfc_t = cpool.tile([P, len(vfc_names) * FC], F32, tag="vfc", name="vfc_t")
        qb_t = cpool.tile([P, NHS], F32, tag="vqb", name="qb_t")
        nc.sync.dma_start(qb_t[:], vec_d["qb"][:])
        vec = {"qb": qb_t}
        for i, k in enumerate(vcc_names):
            nc.sync.dma_start(vcc_t[:, i * CC:(i + 1) * CC], vec_d[k][:])
            vec[k] = vcc_t[:, i * CC:(i + 1) * CC]
        for i, k in enumerate(vfc_names):
            nc.sync.dma_start(vfc_t[:, i * FC:(i + 1) * FC], vec_d[k][:])
            vec[k] = vfc_t[:, i * FC:(i + 1) * FC]
        vb_sb = cpool.tile([1, WVS], BF16, tag="vb", name="vb_sb")
        nc.sync.dma_start(vb_sb[:], vb_e_d[:])
        vbc_sb = cpool.tile([1, WVC], BF16, tag="vbc", name="vbc_sb")
        nc.sync.dma_start(vbc_sb[:], vbc_e_d[:])
        ones_col = cpool.tile([P, 1], BF16, tag="oc", name="ones_col")
        nc.vector.memset(ones_col[:], 1.0)
        ones_row = cpool.tile([1, P], BF16, tag="or", name="ones_row")
        nc.vector.memset(ones_row[:], 1.0)
        eps_t = cpool.tile([1, 1], F32, tag="eps", name="eps_t")
        nc.vector.memset(eps_t[:], float(EPS))

        # ------------- helpers -------------
        def ln_stats(src, nm):
            """src: [P, CC, T] fp32 SBUF. Returns psum broadcasts (rstd_b, mrstd_b)."""
            if src.dtype == BF16:
                xb = src
            else:
                xb = lnp.tile([P, CC, T], BF16, tag="lnxb", bufs=1, name=f"xb_{nm}")
            sq = lnp.tile([P, CC, T], BF16, tag="lnsq", bufs=1, name=f"sq_{nm}")
            for c in range(CC):
                if xb is not src:
                    nc.vector.tensor_copy(xb[:, c], src[:, c])
                nc.vector.tensor_mul(sq[:, c], xb[:, c], xb[:, c])
            s1 = ps_mm.tile([1, T], F32, tag="mm", bufs=2, name=f"s1_{nm}")
            for c in range(CC):
                nc.tensor.matmul(s1[:], ones_col[:], xb[:, c],
                                 start=(c == 0), stop=(c == CC - 1))
            s2 = ps_mm.tile([1, T], F32, tag="mm", bufs=2, name=f"s2_{nm}")
            for c in range(CC):
                nc.tensor.matmul(s2[:], ones_col[:], sq[:, c],
                                 start=(c == 0), stop=(c == CC - 1))
            m = statp.tile([1, T], F32, tag="stat", bufs=5, name=f"m_{nm}")
            nc.vector.tensor_scalar(m[:], s1[:], 1.0 / C, None, ALU.mult)
            ex2 = statp.tile([1, T], F32, tag="stat", bufs=5, name=f"e2_{nm}")
            nc.vector.tensor_scalar(ex2[:], s2[:], 1.0 / C, None, ALU.mult)
            msq = statp.tile([1, T], F32, tag="stat", bufs=5, name=f"ms_{nm}")
            nc.vector.tensor_mul(msq[:], m[:], m[:])
            var = statp.tile([1, T], F32, tag="stat", bufs=5, name=f"va_{nm}")
            nc.vector.tensor_sub(var[:], ex2[:], msq[:])
            sd = statp.tile([1, T], F32, tag="stat", bufs=5, name=f"sd_{nm}")
            nc.scalar.activation(sd[:], var[:], AF.Sqrt, bias=eps_t[:])
            rstd = statp.tile([1, T], F32, tag="stat", bufs=5, name=f"rs_{nm}")
            nc.vector.reciprocal_approx_fast(rstd[:], sd[:])
            mr = statp.tile([1, T], F32, tag="stat", bufs=5, name=f"mr_{nm}")
            nc.vector.tensor_mul(mr[:], m[:], rstd[:])
            rstd_b = statp.tile([1, T], BF16, tag="statb", bufs=2, name=f"rb_{nm}")
            nc.vector.tensor_copy(rstd_b[:], rstd[:])
            mr_b = statp.tile([1, T], BF16, tag="statb", bufs=2, name=f"mb_{nm}")
            nc.vector.tensor_copy(mr_b[:], mr[:])
            bc_r = ps_st.tile([P, 2 * T], F32, tag="st", bufs=2, name=f"bcr_{nm}")
            nc.tensor.matmul(bc_r[:, 0:T], ones_row[:], rstd_b[:], start=True, stop=True)
            nc.tensor.matmul(bc_r[:, T:2 * T], ones_row[:], mr_b[:], start=True, stop=True)
            return bc_r

        def ln_apply(src, bc, g, b, dst, nm):
            """dst[:, c] = ((src*rstd) - m*rstd) * g + b, bf16 out."""
            for c in range(CC):
                u = lnp.tile([P, T], F32, tag="lnu", bufs=4, name=f"u_{nm}{c}")
                nc.vector.tensor_mul(u[:], src[:, c], bc[:, 0:T])
                nc.vector.tensor_sub(u[:], u[:], bc[:, T:2 * T])
                nc.vector.tensor_scalar(dst[:, c], u[:], g[:, c:c + 1], b[:, c:c + 1],
                                        ALU.mult, ALU.add)

        def load_w66(dram, nm, pool=None, htag="w66", parts=P, hdim=CC):
            wp = pool or w66p
            ap = dram if isinstance(dram, bass.AP) else dram[:]
            wt = wp.tile([parts, hdim, ap.shape[-1]], BF16, tag=htag, bufs=2,
                         name=f"w_{nm}")
            nc.sync.dma_start(wt[:], ap.rearrange("(a p) n -> p a n", p=parts))
            return wt

        def linear_fm(dst, src, w_sb, bias, func, nm, cin=CC, dout=CC):
            """dst [P, dout, T] <- act(W^T @ src + bias); w_sb [P, cin, dout*128]."""
            for d in range(dout):
                ps = ps_mm.tile([P, T], F32, tag="mm", bufs=2, name=f"p_{nm}{d}")
                for c in range(cin):
                    nc.tensor.matmul(ps[:], w_sb[:, c, d * P:(d + 1) * P], src[:, c],
                                     start=(c == 0), stop=(c == cin - 1))
                if func is None:
                    nc.scalar.copy(dst[:, d], ps[:])
                else:
                    nc.scalar.activation(dst[:, d], ps[:], func,
                                         bias=bias[:, d:d + 1])

        def linear_resid(dst, src, w_sb, bias, resid, nm, cin=CC):
            for d in range(CC):
                ps = ps_mm.tile([P, T], F32, tag="mm", bufs=2, name=f"pr_{nm}{d}")
                for c in range(cin):
                    nc.tensor.matmul(ps[:], w_sb[:, c, d * P:(d + 1) * P], src[:, c],
                                     start=(c == 0), stop=(c == cin - 1))
                u = lnp.tile([P, T], F32, tag="lnu", bufs=4, name=f"t_{nm}{d}")
                nc.scalar.activation(u[:], ps[:], AF.Identity, bias=bias[:, d:d + 1])
                nc.vector.tensor_add(dst[:, d], u[:], resid[:, d])

        def v_tokmajor(dst, src, wv_sb, vbias, width, nm):
            """dst [P, 4, width] token-major V (+ones cols)."""
            half = width // 2
            for tt in range(4):
                for hh in range(2):
                    ps = ps_mm.tile([P, half], F32, tag="mm", bufs=2,
                                    name=f"v_{nm}{tt}{hh}")
                    for c in range(CC):
                        nc.tensor.matmul(ps[:], src[:, c, tt * P:(tt + 1) * P],
                                         wv_sb[:, c, hh * half:(hh + 1) * half],
                                         start=(c == 0), stop=False)
                    nc.tensor.matmul(ps[:], ones_row[:],
                                     vbias[:, hh * half:(hh + 1) * half],
                                     start=False, stop=True)
                    nc.scalar.copy(dst[:, tt, hh * half:(hh + 1) * half], ps[:])

        def mlp_first(dst, src, w_dram, bias, pool, nm):
            # dst [P, FC, T] = gelu(src @ W1 + b1); W1 [C, FF] streamed in halves
            FH = FC // 2
            for half in range(2):
                wt = pool.tile([P, CC, FH * P], BF16, tag="wma", bufs=2,
                               name=f"wma_{nm}{half}")
                nc.sync.dma_start(
                    wt[:], w_dram[:, half * FH * P:(half + 1) * FH * P]
                    .rearrange("(a p) n -> p a n", p=P))
                for f in range(FH):
                    fo = half * FH + f
                    ps = ps_mm.tile([P, T], F32, tag="mm", bufs=2,
                                    name=f"pm_{nm}{fo}")
                    for c in range(CC):
                        nc.tensor.matmul(ps[:], wt[:, c, f * P:(f + 1) * P],
                                         src[:, c], start=(c == 0),
                                         stop=(c == CC - 1))
                    nc.scalar.activation(dst[:, fo], ps[:], AF.Gelu,
                                         bias=bias[:, fo:fo + 1])

        def mlp_second(dst, src, w_dram, bias, resid, pool, nm):
            # dst [P, CC, T] = src @ W2 + b2 + resid; W2 [FF, C] streamed in halves
            FH = FC // 2
            wts = []
            for half in range(2):
                wt = pool.tile([P, FH, C], BF16, tag="wmb", bufs=2,
                               name=f"wmb_{nm}{half}")
                nc.sync.dma_start(
                    wt[:], w_dram[half * FH * P:(half + 1) * FH * P, :]
                    .rearrange("(a p) n -> p a n", p=P))
                wts.append(wt)
            for d in range(CC):
                ps = ps_mm.tile([P, T], F32, tag="mm", bufs=2, name=f"pr_{nm}{d}")
                for c in range(FC):
                    wt = wts[c // FH]
                    nc.tensor.matmul(ps[:], wt[:, c % FH, d * P:(d + 1) * P],
                                     src[:, c], start=(c == 0), stop=(c == FC - 1))
                u = lnp.tile([P, T], F32, tag="lnu", bufs=4, name=f"t_{nm}{d}")
                nc.scalar.activation(u[:], ps[:], AF.Identity, bias=bias[:, d:d + 1])
                nc.vector.tensor_add(dst[:, d], u[:], resid[:, d])

        def tap(nm, src):
            if KDBG and nm in dbg:
                for c in range(CC):
                    nc.gpsimd.dma_start(
                        dbg[nm][:].rearrange("(a p) n -> p a n", p=P)[:, c], src[:, c])

        # ===================== phase A =====================
        es_x = ExitStack()
        pgx = es_x.enter_context(tc.tile_pool(name="pgx", bufs=1, side="left"))
        es_kv = ExitStack()
        pgkv = es_kv.enter_context(tc.tile_pool(name="pgkv", bufs=1, side="left"))
        es_x1 = ExitStack()
        pgx1 = es_x1.enter_context(tc.tile_pool(name="pgx1", bufs=1, side="right"))
        es_vf = ExitStack()
        pgvf = es_vf.enter_context(tc.tile_pool(name="pgvf", bufs=1, side="right"))
        es_a = ExitStack()
        pga = es_a.enter_context(tc.tile_pool(name="pga", bufs=1, side="left"))

        xT = pgx.tile([P, CC, T], F32, tag="xT", name="xT_sb")
        for c in range(CC):
            nc.sync.dma_start(
                xT[:, c], xT_d[:].rearrange("(a p) n -> p a n", p=P)[:, c])

        h1 = pga.tile([P, CC, T], BF16, tag="h1", name="h1_sb")
        bc = ln_stats(xT, "lx")
        ln_apply(xT, bc, vec["ln1_g"], vec["ln1_b"], h1, "lx")

        KTl = pga.tile([P, CC, T], BF16, tag="KTl", name="KTl_sb")
        wk = load_w66(w_k_d, "wk")
        linear_fm(KTl, h1, wk, vec["kb"], AF.Identity, "kt")
        nc.sync.dma_start(
            kvK_in[:].rearrange("(a p n) -> p a n", p=P, n=T), KTl[:])
        nc.gpsimd.collective_compute(
            "AllGather", ALU.bypass, replica_groups=[list(range(R))],
            ins=[kvK_in[:]], outs=[kvK_out[:]])

        wv = pga.tile([P, CC, WVS], BF16, tag="wv", bufs=1, name="wv_sb")
        nc.sync.dma_start(wv[:], w_ve_d[:].rearrange("(a p) n -> p a n", p=P))
        Vl = pga.tile([P, 4, WVS], BF16, tag="Vl", name="Vl_sb")
        v_tokmajor(Vl, h1, wv, vb_sb, WVS, "vs")
        nc.sync.dma_start(
            kvV_in[:].rearrange("(a p n) -> p a n", p=P, n=WVS), Vl[:])
        nc.gpsimd.collective_compute(
            "AllGather", ALU.bypass, replica_groups=[list(range(R))],
            ins=[kvV_in[:]], outs=[kvV_out[:]])
        Vfull = pgvf.tile([P, NKC, WVS], BF16, tag="Vfull", name="Vfull_sb")
        for r in range(R):
            nc.gpsimd.dma_start(
                Vfull[:, 4 * r:4 * (r + 1), :],
                kvV_out[r, :].rearrange("(a p n) -> p a n", p=P, n=WVS))

        # Q (duplicated across both partition halves for 2-packed scores)
        QT = pgx.tile([P, NHS, T], BF16, tag="QT", name="QT_sb")
        wq0 = load_w66(w_q_d[:, 0:CC * P], "wq0")
        wq1 = load_w66(w_q_d[:, CC * P:2 * CC * P], "wq1")
        for h in range(NHS):
            wt = wq0 if h < CC else wq1
            ps = ps_mm.tile([P, T], F32, tag="mm", bufs=2, name=f"q{h}")
            for c in range(CC):
                nc.tensor.matmul(ps[:], wt[:, c, (h % CC) * P:(h % CC + 1) * P],
                                 h1[:, c], start=(c == 0), stop=(c == CC - 1))
            nc.scalar.activation(QT[:, h], ps[:], AF.Identity,
                                 bias=vec["qb"][:, h:h + 1])

        # cross-attention K/V from y, z (overlaps the AllGather above)
        yT = pga.tile([P, CC, T], BF16, tag="yT", name="yT_sb")
        for c in range(CC):
            nc.sync.dma_start(
                yT[:, c], yT_d[:].rearrange("(a p) n -> p a n", p=P)[:, c])
        hy = pga.tile([P, CC, T], BF16, tag="hyz", bufs=1, name="hy_sb")
        bcy = ln_stats(yT, "ly")
        ln_apply(yT, bcy, vec["ln1_g"], vec["ln1_b"], hy, "ly")
        KcT = pgkv.tile([HDC, NHC, T], BF16, tag="KcT", name="KcT_sb")
        wcak = load_w66(w_cak_d, "wcak")
        for h in range(NHC):
            ps = ps_mm.tile([HDC, T], F32, tag="mm", bufs=2, name=f"kc{h}")
            for c in range(CC):
                nc.tensor.matmul(ps[:], wcak[:, c, HDC * h:HDC * (h + 1)], hy[:, c],
                                 start=(c == 0), stop=(c == CC - 1))
            nc.scalar.copy(KcT[:, h], ps[:])

        zT = pga.tile([P, CC, T], BF16, tag="zT", name="zT_sb")
        for c in range(CC):
            nc.sync.dma_start(
                zT[:, c], zT_d[:].rearrange("(a p) n -> p a n", p=P)[:, c])
        hz = pga.tile([P, CC, T], BF16, tag="hyz", bufs=1, name="hz_sb")
        bcz = ln_stats(zT, "lz")
        ln_apply(zT, bcz, vec["ln1_g"], vec["ln1_b"], hz, "lz")
        wvc = pga.tile([P, CC, WVS], BF16, tag="wv", bufs=1, name="wvc_sb")[:, :, 0:WVC]
        nc.sync.dma_start(wvc[:], w_cave_d[:].rearrange("(a p) n -> p a n", p=P))
        Vcl = pgkv.tile([P, 4, WVC], BF16, tag="Vcl", name="Vcl_sb")
        v_tokmajor(Vcl, hz, wvc, vbc_sb, WVC, "vc")
        tap("d_h1", h1)
        es_a.close()

        # ===================== phase B: self-attention =====================
        es_b = ExitStack()
        pgb = es_b.enter_context(tc.tile_pool(name="pgb", bufs=1, side="right"))
        ktp = pgb
        exp_p = pgb
        atp = pgb

        AT = atp.tile([P, CC, T], BF16, tag="at", name="AT_self")
        sc_s = float(HDS) ** -0.5
        NPR = NKC // 2  # 16 chunk-pairs
        for h in range(NHS):
            # packed K^T: partitions 0-63 = even chunk, 64-127 = odd chunk
            kt = ktp.tile([P, NPR, P], BF16, tag="kt", bufs=2, name=f"ktS{h}")
            for r in range(R):
                src = kvK_out[r, HDS * h * T:(HDS * h + HDS) * T].rearrange(
                    "(p a b n) -> p a b n", p=HDS, a=2, b=2, n=P)
                nc.sync.dma_start(kt[0:HDS, 2 * r:2 * r + 2, :], src[:, :, 0, :])
                nc.sync.dma_start(kt[HDS:P, 2 * r:2 * r + 2, :], src[:, :, 1, :])
            ot = ps_ot.tile([HDS + 1, T], F32, tag="ot", bufs=2, name=f"otS{h}")
            for b2 in range(NPR):
                st = ps_st.tile([P, 2 * T], F32, tag="st", bufs=2, name=f"stS{h}_{b2}")
                nc.tensor.matmul(st[:, 0:T], kt[0:HDS, b2, :], QT[0:HDS, h, :],
                                 start=True, stop=True)
                nc.tensor.matmul(st[:, T:2 * T], kt[HDS:P, b2, :], QT[HDS:P, h, :],
                                 start=True, stop=True)
                ex = exp_p.tile([P, 2 * T], BF16, tag="ex", bufs=4, name=f"exS{h}_{b2}")
                nc.scalar.activation(ex[:], st[:], AF.Exp, scale=sc_s)
                for u in range(2):
                    j = 2 * b2 + u
                    nc.tensor.matmul(ot[:], Vfull[:, j, 65 * h:65 * h + 65],
                                     ex[:, u * T:(u + 1) * T],
                                     start=(j == 0), stop=(j == NKC - 1),
                                     skip_group_check=True)
            denr = statp.tile([1, T], F32, tag="stat", bufs=5, name=f"denrS{h}")
            nc.vector.tensor_copy(denr[:], ot[HDS:HDS + 1, :])
            den = statp.tile([1, T], F32, tag="stat", bufs=5, name=f"denS{h}")
            nc.vector.reciprocal_approx_fast(den[:], denr[:])
            bcd = lnp.tile([HDS, T], F32, tag="bcd", bufs=2, name=f"bcdS{h}")
            nc.gpsimd.partition_broadcast(bcd[:], den[:])
            osb = lnp.tile([HDS, T], F32, tag="osb", bufs=2, name=f"osbS{h}")
            nc.vector.tensor_copy(osb[:], ot[0:HDS, :])
            nc.vector.tensor_mul(AT[HDS * (h % 2):HDS * (h % 2) + HDS, h // 2, :],
                                 osb[:], bcd[:])
            if h == 1:
                # export cross K/V + launch its AllGather while self-attn runs
                nc.sync.dma_start(
                    kvC_in[0:KT_E].rearrange("(p a n) -> p a n", p=HDC, n=T),
                    KcT[:])
                nc.sync.dma_start(
                    kvC_in[KT_E:KVC].rearrange("(a p n) -> p a n", p=P, n=WVC),
                    Vcl[:])
                nc.gpsimd.collective_compute(
                    "AllGather", ALU.bypass, replica_groups=[list(range(R))],
                    ins=[kvC_in[:]], outs=[kvC_out[:]])
                es_kv.close()

        # proj + residual -> x1
        x1 = pgx1.tile([P, CC, T], F32, tag="x1", name="x1_sb")
        wpj = load_w66(w_proj_d, "wpj")
        linear_resid(x1, AT, wpj, vec["projb"], xT, "pj")
        tap("d_x1", x1)
        es_x.close()
        es_b.close()
        es_vf.close()

        # ===================== phase C: MLP2 =====================
        es_x1f = ExitStack()
        pgx1f = es_x1f.enter_context(tc.tile_pool(name="pgx1f", bufs=1, side="left"))
        es_c = ExitStack()
        pgc = es_c.enter_context(tc.tile_pool(name="pgc", bufs=1, side="left"))

        h2 = pgc.tile([P, CC, T], BF16, tag="h2", name="h2_sb")
        bc1 = ln_stats(x1, "l1")
        ln_apply(x1, bc1, vec["ln2_g"], vec["ln2_b"], h2, "l1")

        HT = pgc.tile([P, FC, T], BF16, tag="ht", name="HT2_sb")
        mlp_first(HT, h2, w_m2a_d, vec["m2b1"], pgc, "m2a")
        x1f = pgx1f.tile([P, CC, T], F32, tag="x1f", name="x1f_sb")
        mlp_second(x1f, HT, w_m2b_d, vec["m2b2"], x1, pgc, "m2b")

        for c in range(CC):
            nc.sync.dma_start(
                o_p1[:].rearrange("(a p) n -> p a n", p=P)[:, c], x1f[:, c])
        tap("d_x1f", x1f)
        es_x1.close()
        es_c.close()

        # ===================== phase D: cross-attention =====================
        es_x2 = ExitStack()
        pgx2 = es_x2.enter_context(tc.tile_pool(name="pgx2", bufs=1, side="right"))
        es_d = ExitStack()
        pgd = es_d.enter_context(tc.tile_pool(name="pgd", bufs=1, side="right"))
        ktp = pgd
        exp_p = pgd
        atp = pgd

        hq = pgd.tile([P, CC, T], BF16, tag="hq", name="hq_sb")
        bcq = ln_stats(x1f, "lq")
        ln_apply(x1f, bcq, vec["ln1_g"], vec["ln1_b"], hq, "lq")

        QcT = pgd.tile([HDC, NHC, T], BF16, tag="QcT", name="QcT_sb")
        wcaq = load_w66(w_caq_d, "wcaq")
        for h in range(NHC):
            ps = ps_mm.tile([HDC, T], F32, tag="mm", bufs=2, name=f"qc{h}")
            for c in range(CC):
                nc.tensor.matmul(ps[:], wcaq[:, c, HDC * h:HDC * (h + 1)], hq[:, c],
                                 start=(c == 0), stop=(c == CC - 1))
            nc.vector.tensor_copy(QcT[:, h], ps[:])

        Vcfull = pgd.tile([P, NKC, WVC], BF16, tag="Vcfull", name="Vcfull_sb")
        for r in range(R):
            nc.gpsimd.dma_start(
                Vcfull[:, 4 * r:4 * (r + 1), :],
                kvC_out[r, KT_E:KVC].rearrange("(a p n) -> p a n", p=P, n=WVC))

        AcT = atp.tile([HDC, NHC, T], BF16, tag="atc", name="AT_cross")
        sc_c = float(HDC) ** -0.5
        for h in range(NHC):
            kt = ktp.tile([HDC, NKC, P], BF16, tag="kt", bufs=2, name=f"ktC{h}")
            for r in range(R):
                nc.sync.dma_start(
                    kt[:, 4 * r:4 * (r + 1), :],
                    kvC_out[r, 0:KT_E]
                    .rearrange("(p a n) -> p a n", p=HDC, n=T)[:, h, :]
                    .rearrange("p (j n) -> p j n", n=P))
            qrhs = QcT[:, h, :]
            ot = ps_ot.tile([HDC + 1, T], F32, tag="ot", bufs=2, name=f"otC{h}")
            for b2 in range(NKC // 2):
                st = ps_st.tile([P, 2 * T], F32, tag="st", bufs=2, name=f"stC{h}_{b2}")
                for u in range(2):
                    j = 2 * b2 + u
                    nc.tensor.matmul(st[:, u * T:(u + 1) * T], kt[:, j, :], qrhs,
                                     start=True, stop=True)
                ex = exp_p.tile([P, 2 * T], BF16, tag="ex", bufs=4, name=f"exC{h}_{b2}")
                nc.scalar.activation(ex[:], st[:], AF.Exp, scale=sc_c)
                for u in range(2):
                    j = 2 * b2 + u
                    nc.tensor.matmul(ot[:], Vcfull[:, j, 97 * h:97 * h + 97],
                                     ex[:, u * T:(u + 1) * T],
                                     start=(j == 0), stop=(j == NKC - 1),
                                     skip_group_check=True)
            denr = statp.tile([1, T], F32, tag="stat", bufs=5, name=f"denrC{h}")
            nc.vector.tensor_copy(denr[:], ot[HDC:HDC + 1, :])
            den = statp.tile([1, T], F32, tag="stat", bufs=5, name=f"denC{h}")
            nc.vector.reciprocal_approx_fast(den[:], denr[:])
            bcd = lnp.tile([HDC, T], F32, tag="bcd", bufs=2, name=f"bcdC{h}")
            nc.gpsimd.partition_broadcast(bcd[:], den[:])
            osb = lnp.tile([HDC, T], F32, tag="osb", bufs=2, name=f"osbC{h}")
            nc.vector.tensor_copy(osb[:], ot[0:HDC, :])
            nc.vector.tensor_mul(AcT[:, h, :], osb[:], bcd[:])

        # ca_o + residual -> x2
        x2 = pgx2.tile([P, CC, T], F32, tag="x2", name="x2_sb")
        wcao = pgd.tile([HDC, NHC, C], BF16, tag="wcao", name="wcao_sb")
        nc.sync.dma_start(wcao[:], w_cao_d[:].rearrange("(a p) n -> p a n", p=HDC))
        for d in range(CC):
            ps = ps_mm.tile([P, T], F32, tag="mm", bufs=2, name=f"cao{d}")
            for h in range(NHC):
                nc.tensor.matmul(ps[:], wcao[:, h, d * P:(d + 1) * P], AcT[:, h, :],
                                 start=(h == 0), stop=(h == NHC - 1))
            u = lnp.tile([P, T], F32, tag="lnu", bufs=4, name=f"tcao{d}")
            nc.scalar.activation(u[:], ps[:], AF.Identity, bias=vec["caob"][:, d:d + 1])
            nc.vector.tensor_add(x2[:, d], u[:], x1f[:, d])
        tap("d_x2", x2)
        es_x1f.close()
        es_d.close()

        # ===================== phase E: MLP + pw heads =====================
        es_e = ExitStack()
        pge = es_e.enter_context(tc.tile_pool(name="pge", bufs=1, side="left"))

        h3 = pge.tile([P, CC, T], BF16, tag="h3", name="h3_sb")
        bc2 = ln_stats(x2, "l2")
        ln_apply(x2, bc2, vec["ln2_g"], vec["ln2_b"], h3, "l2")

        HT1 = pge.tile([P, FC, T], BF16, tag="ht", name="HT1_sb")
        mlp_first(HT1, h3, w_m1a_d, vec["m1b1"], pge, "m1a")
        p2 = pge.tile([P, CC, T], F32, tag="p2", name="p2_sb")
        mlp_second(p2, HT1, w_m1b_d, vec["m1b2"], x2, pge, "m1b")
        tap("d_p2", p2)

        p2b = lnp.tile([P, CC, T], BF16, tag="lnxb", bufs=1, name="p2b_sb")
        for c in range(CC):
            nc.vector.tensor_copy(p2b[:, c], p2[:, c])

        for w_d, bias, out_d, nm in ((w_pw1_d, "pw1b", o_pw1, "pw1"),
                                     (w_pw2_d, "pw2b", o_pw2, "pw2")):
            wt = load_w66(w_d, nm)
            for d in range(CC):
                ps = ps_mm.tile([P, T], F32, tag="mm", bufs=2, name=f"p_{nm}{d}")
                for c in range(CC):
                    nc.tensor.matmul(ps[:], wt[:, c, d * P:(d + 1) * P], p2b[:, c],
                                     start=(c == 0), stop=(c == CC - 1))
                u = lnp.tile([P, T], F32, tag="lnu", bufs=4, name=f"o_{nm}{d}")
                nc.scalar.activation(u[:], ps[:], AF.Gelu, bias=vec[bias][:, d:d + 1])
                nc.sync.dma_start(
                    out_d[:].rearrange("(a p) n -> p a n", p=P)[:, d], u[:])
        es_x2.close()
        es_e.close()

    nc.finalize()
    return nc


def _prep_inputs(inputs):
    f32 = np.float32

    def bf(a):
        return np.ascontiguousarray(a).astype(BFNP)

    def vec128(v, w):
        return np.ascontiguousarray(np.asarray(v, f32).reshape(w, P).T)

    x = np.asarray(inputs["x"], f32).reshape(N, C)
    y = np.asarray(inputs["y"], f32).reshape(N, C)
    z = np.asarray(inputs["z"], f32).reshape(N, C)
    xT = np.ascontiguousarray(x.T)
    yT = np.ascontiguousarray(y.T)
    zT = np.ascontiguousarray(z.T)

    qkv_w = np.asarray(inputs["qkv_w"], f32)
    qkv_b = np.asarray(inputs["qkv_b"], f32)
    w_q = np.zeros((C, NHS * P), f32)
    for h in range(NHS):
        w_q[:, P * h:P * h + HDS] = qkv_w[:, HDS * h:HDS * h + HDS]
        w_q[:, P * h + HDS:P * (h + 1)] = qkv_w[:, HDS * h:HDS * h + HDS]
    w_q = bf(w_q)
    w_k = bf(qkv_w[:, C:2 * C])
    w_v = qkv_w[:, 2 * C:3 * C]
    w_ve = np.zeros((C, WVS), f32)
    vb_e = np.zeros((1, WVS), f32)
    for h in range(NHS):
        w_ve[:, 65 * h:65 * h + 64] = w_v[:, 64 * h:64 * h + 64]
        vb_e[0, 65 * h:65 * h + 64] = qkv_b[2 * C + 64 * h:2 * C + 64 * h + 64]
        vb_e[0, 65 * h + 64] = 1.0

    ca_v = np.asarray(inputs["ca_v_w"], f32)
    w_cave = np.zeros((C, WVC), f32)
    vbc_e = np.zeros((1, WVC), f32)
    for h in range(NHC):
        w_cave[:, 97 * h:97 * h + 96] = ca_v[:, 96 * h:96 * h + 96]
        vbc_e[0, 97 * h + 96] = 1.0

    common = {
        "w_q": w_q, "w_k": w_k, "w_ve": bf(w_ve), "vb_e": bf(vb_e),
        "w_proj": bf(inputs["proj_w"]),
        "w_caq": bf(inputs["ca_q_w"]), "w_cak": bf(inputs["ca_k_w"]),
        "w_cave": bf(w_cave), "vbc_e": bf(vbc_e),
        "w_cao": bf(inputs["ca_o_w"]),
        "w_m2a": bf(inputs["mlp2_w1"]), "w_m2b": bf(inputs["mlp2_w2"]),
        "w_m1a": bf(inputs["mlp_w1"]), "w_m1b": bf(inputs["mlp_w2"]),
        "ln1_g": vec128(inputs["ln1_g"], CC), "ln1_b": vec128(inputs["ln1_b"], CC),
        "ln2_g": vec128(inputs["ln2_g"], CC), "ln2_b": vec128(inputs["ln2_b"], CC),
        "qb": np.ascontiguousarray(np.tile(
            np.asarray(qkv_b[0:C], f32).reshape(NHS, HDS), 2).T),
        "kb": vec128(qkv_b[C:2 * C], CC),
        "projb": vec128(inputs["proj_b"], CC), "caob": vec128(inputs["ca_o_b"], CC),
        "m2b1": vec128(inputs["mlp2_b1"], FC), "m2b2": vec128(inputs["mlp2_b2"], CC),
        "m1b1": vec128(inputs["mlp_b1"], FC), "m1b2": vec128(inputs["mlp_b2"], CC),
    }
    for nm in ("pw1", "pw2"):
        w = np.asarray(inputs[nm + "_w"], f32)
        g = np.asarray(inputs[nm + "_bn_g"], f32)
        b = np.asarray(inputs[nm + "_bn_b"], f32)
        m = np.asarray(inputs[nm + "_bn_m"], f32)
        v = np.asarray(inputs[nm + "_bn_v"], f32)
        scale = g / np.sqrt(v + EPS)
        common["w_" + nm] = bf((w * scale[:, None]).T)
        common[nm + "b"] = vec128(b - m * scale, CC)

    in_maps = []
    for r in range(R):
        m_ = dict(common)
        m_["xT"] = np.ascontiguousarray(xT[:, r * T:(r + 1) * T])
        m_["yT"] = np.ascontiguousarray(yT[:, r * T:(r + 1) * T]).astype(BFNP)
        m_["zT"] = np.ascontiguousarray(zT[:, r * T:(r + 1) * T]).astype(BFNP)
        in_maps.append(m_)
    return in_maps


def _run(inputs, trace=False):
    global _BUILT
    if _BUILT is None:
        _BUILT = _build()
    nc = _BUILT
    in_maps = _prep_inputs(inputs)
    res = run_bass_kernel_spmd(nc, in_maps, core_ids=list(range(R)), trace=trace)

    def gather(name):
        full = np.concatenate([res.results[r][name] for r in range(R)], axis=1)
        return np.ascontiguousarray(full.T).reshape(1, 64, 64, C)

    outs = (gather("o_p1"), gather("o_pw1"), gather("o_pw2"))
    return outs, res


def kernel(**inputs):
    outs, _ = _run(inputs, trace=False)
    return outs
